# revision 10
# baseline (speedup 1.0000x reference)
import sys
import numpy as np

if "/opt/trn_rl_repo" not in sys.path:
    sys.path.insert(0, "/opt/trn_rl_repo")

B, S, D, H, DK = 8, 1024, 512, 8, 64
SQ = S // 128      # 8 s-tiles of 128
DC = D // 128      # 4 d-chunks of 128
NEG = -9.0e15
SCALE = 32.0       # sqrt(len_k) = sqrt(1024)

_CACHE = {}


def _build_module():
    from contextlib import ExitStack
    from concourse import bacc, tile, mybir
    from concourse.masks import make_identity

    f32 = mybir.dt.float32
    bf16 = mybir.dt.bfloat16
    f16 = mybir.dt.float16
    i32 = mybir.dt.int32
    AF = mybir.ActivationFunctionType
    OP = mybir.AluOpType
    AX = mybir.AxisListType

    nc = bacc.Bacc(
        "TRN2",
        target_bir_lowering=False,
        debug=False,
        enable_asserts=False,
        num_devices=B,
    )

    q_d = nc.dram_tensor("q", [S, D], f32, kind="ExternalInput").ap()
    k_d = nc.dram_tensor("k", [S, D], f32, kind="ExternalInput").ap()
    v_d = nc.dram_tensor("v", [S, D], f32, kind="ExternalInput").ap()
    adj_d = nc.dram_tensor("adj", [S, S], i32, kind="ExternalInput").ap()
    wq_d = nc.dram_tensor("w_qs", [H, D, DK], f32, kind="ExternalInput").ap()
    wk_d = nc.dram_tensor("w_ks", [H, D, DK], f32, kind="ExternalInput").ap()
    wv_d = nc.dram_tensor("w_vs", [H, D, DK], f32, kind="ExternalInput").ap()
    pw_d = nc.dram_tensor("proj_w", [D, D], f32, kind="ExternalInput").ap()
    pb_d = nc.dram_tensor("proj_b", [1, D], f32, kind="ExternalInput").ap()
    out_d = nc.dram_tensor("out", [S, D], f32, kind="ExternalOutput").ap()
    attns_d = nc.dram_tensor("attns", [H, S, S], f32, kind="ExternalOutput").ap()

    with tile.TileContext(nc) as tc, ExitStack() as ctx:
        persist = ctx.enter_context(tc.tile_pool(name="persist", bufs=1))

        ident = persist.tile([128, 128], f32, name="ident")
        make_identity(nc, ident)
        ones_sb = persist.tile([1, 128], f32, name="ones_sb")
        nc.gpsimd.memset(ones_sb, 1.0)
        pb_sb = persist.tile([1, D], f32, name="pb_sb")
        nc.sync.dma_start(pb_sb, pb_d[:, :])

        # persistent operands
        q_sT = [persist.tile([128, S], f32, name=f"q_sT{g}") for g in range(4)]
        k_sT = [persist.tile([128, S], f32, name=f"k_sT{g}") for g in range(4)]
        v_s = [persist.tile([128, 512], f16, name=f"v_s{i}") for i in range(SQ)]
        negmask = [persist.tile([128, S], bf16, name=f"negmask{i}") for i in range(SQ)]
        out_cat = [persist.tile([128, 512], f32, name=f"out_cat{i}") for i in range(SQ)]
        proj_wT = [persist.tile([128, 512], f32, name=f"proj_wT{c}") for c in range(DC)]

        # ---- adj -> negmask (bf16): 0 where adj>0 else -32*9e15 ----
        NEGK = SCALE * NEG
        with tc.tile_pool(name="adj_stage", bufs=2) as apool:
            for i in range(SQ):
                adj_t = apool.tile([128, S], i32, tag="adj")
                nc.gpsimd.dma_start(adj_t, adj_d[i * 128:(i + 1) * 128, :])
                nc.vector.tensor_scalar(
                    negmask[i], adj_t, -NEGK, NEGK, op0=OP.mult, op1=OP.add
                )

        # ---- per-tensor: load, transpose, project ----
        def load_transpose(name, src_d, sp, pp):
            nat = [sp.tile([128, D], f32, name=f"{name}_nat{i}") for i in range(SQ)]
            for i in range(SQ):
                nc.sync.dma_start(nat[i], src_d[i * 128:(i + 1) * 128, :])
            tT = [sp.tile([128, S], f32, name=f"{name}_T{c}") for c in range(DC)]
            for c in range(DC):
                for half in range(2):
                    ps = pp.tile([128, 512], f32, tag="tp")
                    for j in range(4):
                        si = half * 4 + j
                        nc.tensor.transpose(
                            ps[:, j * 128:(j + 1) * 128],
                            nat[si][:, c * 128:(c + 1) * 128],
                            ident,
                        )
                    nc.scalar.copy(tT[c][:, half * 512:(half + 1) * 512], ps)
            return tT

        def load_weights(name, w_d, sp):
            w_sb = [sp.tile([128, 512], f32, name=f"{name}_w{c}") for c in range(DC)]
            for c in range(DC):
                for h in range(H):
                    nc.scalar.dma_start(
                        w_sb[c][:, h * 64:(h + 1) * 64],
                        w_d[h, c * 128:(c + 1) * 128, :],
                    )
            return w_sb

        for name, src_d, w_d in (("q", q_d, wq_d), ("k", k_d, wk_d), ("v", v_d, wv_d)):
            with tc.tile_pool(name=f"{name}_stage", bufs=1) as sp, \
                 tc.tile_pool(name=f"{name}_ps", bufs=2, space="PSUM") as pp, \
                 tc.tile_pool(name=f"{name}_ps2", bufs=2, space="PSUM") as pp2:
                w_sb = load_weights(name, w_d, sp)
                tT = load_transpose(name, src_d, sp, pp)
                if name in ("q", "k"):
                    dst = q_sT if name == "q" else k_sT
                    for g in range(4):
                        for sh in range(2):
                            ps2 = pp2.tile([128, 512], f32, tag="pj")
                            for c in range(DC):
                                nc.tensor.matmul(
                                    ps2,
                                    w_sb[c][:, g * 128:(g + 1) * 128],
                                    tT[c][:, sh * 512:(sh + 1) * 512],
                                    start=(c == 0),
                                    stop=(c == DC - 1),
                                )
                            nc.scalar.copy(dst[g][:, sh * 512:(sh + 1) * 512], ps2)
                else:
                    for si in range(SQ):
                        ps2 = pp2.tile([128, 512], f32, tag="pj")
                        for c in range(DC):
                            nc.tensor.matmul(
                                ps2,
                                tT[c][:, si * 128:(si + 1) * 128],
                                w_sb[c],
                                start=(c == 0),
                                stop=(c == DC - 1),
                            )
                        nc.scalar.copy(v_s[si], ps2)

        # ---- proj_w -> proj_wT ----
        with tc.tile_pool(name="pw_stage", bufs=1) as pwp, \
             tc.tile_pool(name="pw_ps", bufs=2, space="PSUM") as pp3:
            pw_nat = [pwp.tile([128, D], f32, name=f"pw{c}") for c in range(DC)]
            for c in range(DC):
                nc.sync.dma_start(pw_nat[c], pw_d[c * 128:(c + 1) * 128, :])
            for cj in range(DC):
                ps = pp3.tile([128, 512], f32, tag="pwt")
                for cd in range(DC):
                    nc.tensor.transpose(
                        ps[:, cd * 128:(cd + 1) * 128],
                        pw_nat[cd][:, cj * 128:(cj + 1) * 128],
                        ident,
                    )
                nc.scalar.copy(proj_wT[cj], ps)

        # ---- stage C: attention per (qi, h), software-pipelined ----
        iters = [(qi, h) for qi in range(SQ) for h in range(H)]
        with tc.tile_pool(name="c_sb", bufs=2) as csb, \
             tc.tile_pool(name="c_sc", bufs=2) as csc, \
             tc.tile_pool(name="ps_s", bufs=2, space="PSUM") as pss, \
             tc.tile_pool(name="ps_t", bufs=1, space="PSUM") as pst_pool, \
             tc.tile_pool(name="ps_v", bufs=2, space="PSUM") as psv:
            state = None
            for idx in range(len(iters) + 1):
                cur = None
                if idx < len(iters):
                    qi, h = iters[idx]
                    g, ho = h // 2, (h % 2) * 64
                    ps = pss.tile([128, S], f32, tag="scores")
                    for kh in range(2):
                        nc.tensor.matmul(
                            ps[:, kh * 512:(kh + 1) * 512],
                            q_sT[g][ho:ho + 64, qi * 128:(qi + 1) * 128],
                            k_sT[g][ho:ho + 64, kh * 512:(kh + 1) * 512],
                            start=True,
                            stop=True,
                        )
                    smask = csb.tile([128, S], f32, tag="smask")
                    nc.vector.scalar_tensor_tensor(
                        out=smask,
                        in0=ps,
                        scalar=SCALE,
                        in1=negmask[qi],
                        op0=OP.mult,
                        op1=OP.add,
                    )
                    m32 = csc.tile([128, 1], f32, tag="m32")
                    nc.vector.tensor_reduce(
                        m32, smask, axis=AX.X, op=OP.max, negate=True
                    )
                    p = csb.tile([128, S], f32, tag="p")
                    ssum = csc.tile([128, 1], f32, tag="ssum")
                    nc.scalar.activation(
                        p, smask, AF.Exp, bias=m32, scale=1.0, accum_out=ssum
                    )
                    r = csc.tile([128, 1], f32, tag="r")
                    nc.vector.reciprocal(r, ssum)
                    alpha = csb.tile([128, S], f32, tag="alpha", bufs=3)
                    nc.gpsimd.tensor_scalar(alpha, p, r, None, op0=OP.mult)
                    dma_eng = nc.sync if (idx % 2 == 0) else nc.scalar
                    dma_eng.dma_start(attns_d[h, qi * 128:(qi + 1) * 128, :], alpha)
                    cur = (p, r, qi, h)
                if state is not None:
                    p_, r_, qi_, h_ = state
                    pst = pst_pool.tile([128, S], f32, tag="pt")
                    for ki in range(SQ):
                        nc.tensor.transpose(
                            pst[:, ki * 128:(ki + 1) * 128],
                            p_[:, ki * 128:(ki + 1) * 128],
                            ident,
                        )
                    pT = csb.tile([128, S], f16, tag="pT")
                    nc.scalar.copy(pT, pst)
                    pv = psv.tile([128, DK], f32, tag="pv")
                    for ki in range(SQ):
                        nc.tensor.matmul(
                            pv,
                            pT[:, ki * 128:(ki + 1) * 128],
                            v_s[ki][:, h_ * 64:(h_ + 1) * 64],
                            start=(ki == 0),
                            stop=(ki == SQ - 1),
                        )
                    nc.vector.tensor_scalar(
                        out_cat[qi_][:, h_ * 64:(h_ + 1) * 64],
                        pv,
                        r_,
                        None,
                        op0=OP.mult,
                    )
                state = cur

        # ---- stage D: concat-transpose + final projection ----
        with tc.tile_pool(name="d_sb", bufs=1) as dsb, \
             tc.tile_pool(name="d_ps", bufs=2, space="PSUM") as dps, \
             tc.tile_pool(name="d_ps2", bufs=2, space="PSUM") as dps2:
            catT = [dsb.tile([128, S], f32, name=f"catT{c}") for c in range(DC)]
            for c in range(DC):
                for half in range(2):
                    ps = dps.tile([128, 512], f32, tag="dt")
                    for j in range(4):
                        qi = half * 4 + j
                        nc.tensor.transpose(
                            ps[:, j * 128:(j + 1) * 128],
                            out_cat[qi][:, c * 128:(c + 1) * 128],
                            ident,
                        )
                    nc.scalar.copy(catT[c][:, half * 512:(half + 1) * 512], ps)
            for qi in range(SQ):
                psf = dps2.tile([128, 512], f32, tag="fin")
                for c in range(DC):
                    nc.tensor.matmul(
                        psf,
                        catT[c][:, qi * 128:(qi + 1) * 128],
                        proj_wT[c],
                        start=(c == 0),
                        stop=False,
                    )
                nc.tensor.matmul(psf, ones_sb, pb_sb, start=False, stop=True)
                out_sb = dsb.tile([128, 512], f32, tag="osb", bufs=2)
                nc.scalar.copy(out_sb, psf)
                nc.sync.dma_start(out_d[qi * 128:(qi + 1) * 128, :], out_sb)

    nc.compile()
    return nc


def _get_module():
    if "nc" not in _CACHE:
        _CACHE["nc"] = _build_module()
    return _CACHE["nc"]


def _time_warm_runs(nc, in_maps, n_cores, n_iters=5):
    """Wall-clock the warm sharded executable (inputs device-resident,
    zero output buffers made device-side) and return (min_ns, results)."""
    import time as _time

    import jax
    import jax.numpy as jnp
    from jax.sharding import Mesh, NamedSharding, PartitionSpec
    from jax.experimental.shard_map import shard_map

    from concourse import mybir
    from concourse.bass2jax import (
        _bass_exec_p,
        install_neuronx_cc_hook,
        partition_id_tensor,
    )

    install_neuronx_cc_hook()

    partition_name = (
        nc.partition_id_tensor.name if nc.partition_id_tensor else None
    )
    in_names, out_names, out_avals = [], [], []
    for alloc in nc.m.functions[0].allocations:
        if not isinstance(alloc, mybir.MemoryLocationSet):
            continue
        name = alloc.memorylocations[0].name
        if alloc.kind == "ExternalInput":
            if name != partition_name:
                in_names.append(name)
        elif alloc.kind == "ExternalOutput":
            shape = tuple(alloc.tensor_shape)
            dtype = mybir.dt.np(alloc.dtype)
            out_names.append(name)
            out_avals.append(jax.core.ShapedArray(shape, dtype))
    n_params = len(in_names)
    n_outs = len(out_avals)
    all_names = list(in_names) + list(out_names)
    if partition_name is not None:
        all_names.append(partition_name)
    donate = tuple(range(n_params, n_params + n_outs))

    def _body(*args):
        operands = list(args)
        if partition_name is not None:
            operands.append(partition_id_tensor())
        return tuple(
            _bass_exec_p.bind(
                *operands,
                out_avals=tuple(out_avals),
                in_names=tuple(all_names),
                out_names=tuple(out_names),
                lowering_input_output_aliases=(),
                sim_require_finite=True,
                sim_require_nnan=True,
                nc=nc,
            )
        )

    devices = jax.devices()[:n_cores]
    mesh = Mesh(np.asarray(devices), ("core",))
    spec = PartitionSpec("core")
    sharding = NamedSharding(mesh, spec)
    sharded = jax.jit(
        shard_map(
            _body,
            mesh=mesh,
            in_specs=(spec,) * (n_params + n_outs),
            out_specs=(spec,) * n_outs,
            check_rep=False,
        ),
        donate_argnums=donate,
        keep_unused=True,
    )
    concat_in = [
        np.concatenate([np.asarray(m[name])[None] for m in in_maps], axis=0).reshape(
            n_cores * np.asarray(in_maps[0][name]).shape[0],
            *np.asarray(in_maps[0][name]).shape[1:],
        )
        for name in in_names
    ]
    dev_in = [jax.device_put(x, sharding) for x in concat_in]
    zero_shapes = [
        ((n_cores * a.shape[0], *a.shape[1:]), a.dtype) for a in out_avals
    ]
    make_zeros = jax.jit(
        lambda: tuple(jnp.zeros(s, d) for s, d in zero_shapes),
        out_shardings=(sharding,) * n_outs,
    )

    # warm-up: first call compiles/loads NEFF onto all cores
    zs = make_zeros()
    outs = sharded(*dev_in, *zs)
    jax.block_until_ready(outs)
    results = [
        {
            name: np.asarray(outs[i]).reshape(n_cores, *out_avals[i].shape)[c]
            for i, name in enumerate(out_names)
        }
        for c in range(n_cores)
    ]

    times = []
    for _ in range(n_iters):
        zs = make_zeros()
        jax.block_until_ready(zs)
        t0 = _time.perf_counter()
        o = sharded(*dev_in, *zs)
        jax.block_until_ready(o)
        times.append(_time.perf_counter() - t0)
    return int(min(times) * 1e9), results


def kernel(**inputs):
    import os

    nc = _get_module()

    def f32c(x):
        return np.ascontiguousarray(np.asarray(x, dtype=np.float32))

    q = f32c(inputs["q"])
    k = f32c(inputs["k"])
    v = f32c(inputs["v"])
    adj = np.ascontiguousarray(np.asarray(inputs["adj"], dtype=np.int32))
    shared = {
        "w_qs": f32c(inputs["w_qs"]),
        "w_ks": f32c(inputs["w_ks"]),
        "w_vs": f32c(inputs["w_vs"]),
        "proj_w": f32c(inputs["proj_w"]),
        "proj_b": f32c(inputs["proj_b"]).reshape(1, D),
    }
    in_maps = [
        dict(q=q[b], k=k[b], v=v[b], adj=adj[b], **shared) for b in range(B)
    ]
    from concourse.bass_utils import run_bass_kernel_spmd

    res = run_bass_kernel_spmd(nc, in_maps, list(range(B)), trace=False)
    results = res.results
    _CACHE["exec_time_ns"] = getattr(res, "exec_time_ns", None)
    if _CACHE["exec_time_ns"] is None and not os.environ.get("KNOTIME"):
        try:
            ns, _ = _time_warm_runs(nc, in_maps, B)
            _CACHE["exec_time_ns"] = ns
        except Exception as e:
            _CACHE["time_err"] = repr(e)
    out = np.stack([results[b]["out"] for b in range(B)], axis=0)
    attns = np.stack([results[b]["attns"] for b in range(B)], axis=1)
    return out, attns
